# revision 49
# baseline (speedup 1.0000x reference)
"""T5-style encoder self-attention (dense_transformer) on 8 Trainium2 NeuronCores.

Problem (full shapes): hidden [2,2048,2048], Wq/Wk/Wv/Wo [2048,2048],
rel_emb [32,32] (bidirectional T5 relative-position bias), mask [2,1,1,2048].

Sharding: data-parallel over batch (2) x tensor-parallel over heads (4 groups
of 8 heads) = 8 cores, Megatron-style. Each core computes a partial output
[2048,2048] for its batch (its 8 heads through its Wo row-slice); the host
sums 4 partials per batch.

Per-core kernel design (bf16 operands, fp32 PSUM accumulation):
  - x^T streamed as per-chunk SLABS [128, 16, 512] (one DMA each) so the DMA
    engines stay ahead of the PE; phase 1 does pair-0 Q/K AND V from the same
    slab pass.
  - Q^T stored s-REVERSED so the relative-position bias becomes a
    positive-shear Toeplitz U_h[p,j] = exp(bias_h)[diag p+j-2047], built with
    one contiguous sheared DMA per head from a device-built table.
  - scores^T tiles [k=128part, q=512free]: row-packed pair of K=64 matmuls
    (tile_position (0,0)/(64,0)) computes 2 heads into one [128,1024] PSUM
    tile; ONE ACT exp covers both heads; ONE fused DVE multiply applies both
    heads' Toeplitz exp-bias (u tiles packed per-pair).
  - PV with V_aug=[V | ones-col] (M=65): psum row 64 accumulates the softmax
    denominator for free.
  - normalization is local: DVE reciprocal straight off the PSUM denominator
    row, a K=1 broadcast matmul replicates it across 128 partitions, one DVE
    multiply rescales ctxt. No DRAM roundtrips.
  - the PE queue is kept saturated by emitting, inside EVERY attention kt
    iteration, 2 independent filler matmuls: the next pair's Q/K projection
    (pairs 0-2) or the output projection of already-normalized q-ranges
    (pair 3). Exp/multiply latency hides behind them, which also keeps the
    HAM clock gate at full rate.
  - output projection reuses one ctxt weight load for 2 consecutive matmuls
    (nd pairs) and drains through a 3-tag PSUM rotation in the tail.
"""

import math
import sys

for _p in ("/opt/trn_rl_repo",):
    if _p not in sys.path:
        sys.path.insert(0, _p)

import numpy as np

import concourse.bass as bass
import concourse.mybir as mybir
import concourse.tile as tile
from concourse import bacc
from concourse.bass_utils import run_bass_kernel_spmd

DT = mybir.dt
AF = mybir.ActivationFunctionType
OP = mybir.AluOpType

# ---- problem constants (hardcoded per contract) ----
B, S, D = 2, 2048, 2048
N_HEADS, D_KV = 32, 64
NUM_BUCKETS, MAX_DISTANCE = 32, 128
NCORES = 8
HL = 8            # heads per core
P = 128
SC = 512          # free-dim chunk
NKT = S // P      # 16 k-tiles
NQC = S // SC     # 4 q-chunks
NDT = D // P      # 16 D-tiles
NMT = (HL * D_KV) // P   # 4 hd m-tiles per core
MV = D_KV + 1     # PV stationary width: 64 V cols + 1 ones col (denominator)
W_U = 3968        # toeplitz tile width: max j0 (=15*128+3*512) + 512
NDIAG = 4096      # ediag row stride (4095 used)


def _rel_bucket_host(d):
    """Exact numpy replica of reference._relative_position_bucket (fp32 math,
    int32 truncation) for bidirectional buckets. d = k - q (int array)."""
    num_buckets = NUM_BUCKETS // 2          # 16
    max_exact = num_buckets // 2            # 8
    rel = np.asarray(d, dtype=np.int64)
    buckets = (rel > 0).astype(np.int32) * num_buckets
    arel = np.abs(rel)
    is_small = arel < max_exact
    rp_safe = np.maximum(arel, 1).astype(np.float32)
    log_ratio = np.log(rp_safe / np.float32(max_exact)).astype(np.float32)
    scale = np.float32(math.log(MAX_DISTANCE / max_exact))
    rp_large = max_exact + (log_ratio / scale * np.float32(num_buckets - max_exact)).astype(np.int32)
    rp_large = np.minimum(rp_large, num_buckets - 1)
    buckets = buckets + np.where(is_small, arel.astype(np.int32), rp_large)
    return buckets.astype(np.int32)


def _onehot_const():
    """OH[u, i] = 1 if bucket(i - 2047) == u, i in [0, 4095); col 4095 = 0."""
    i = np.arange(NDIAG - 1)
    b = _rel_bucket_host(i - (S - 1))
    oh = np.zeros((NUM_BUCKETS, NDIAG), dtype=np.float32)
    oh[b, i] = 1.0
    return oh


def _rev_free(ap_slice, width):
    """Reversed-free-dim view of a 2D [parts, width] AP slice."""
    return bass.AP(
        tensor=ap_slice.tensor,
        offset=ap_slice.offset + (width - 1),
        ap=[list(ap_slice.ap[0]), [-1, width]],
    )


def _build():
    nc = bacc.Bacc(None, name="attn_tp")

    xt = nc.declare_dram_parameter("xt", [D, S], DT.bfloat16, isOutput=False)
    wq = nc.declare_dram_parameter("wq", [D, HL * D_KV], DT.bfloat16, isOutput=False)
    wk = nc.declare_dram_parameter("wk", [D, HL * D_KV], DT.bfloat16, isOutput=False)
    wv = nc.declare_dram_parameter("wv", [D, HL * D_KV], DT.bfloat16, isOutput=False)
    wo = nc.declare_dram_parameter("wo", [HL * D_KV, D], DT.bfloat16, isOutput=False)
    mask = nc.declare_dram_parameter("mask", [S], DT.float32, isOutput=False)
    rel = nc.declare_dram_parameter("rel", [NUM_BUCKETS, HL], DT.float32, isOutput=False)
    oh = nc.declare_dram_parameter("oh", [NUM_BUCKETS, NDIAG], DT.float32, isOutput=False)
    out = nc.declare_dram_parameter("out", [S, D], DT.float32, isOutput=True)

    with tile.TileContext(nc) as tc:
        with (
            tc.tile_pool(name="res", bufs=1) as res,          # persistent tensors
            tc.tile_pool(name="xsl", bufs=2) as xsl,          # x^T slabs
            tc.tile_pool(name="ubig", bufs=2) as ubig,        # u pairs / wo
            tc.tile_pool(name="stage", bufs=2) as stage,      # small staging
            tc.tile_pool(name="pexp", bufs=3) as pexpp,       # probs tiles
            tc.tile_pool(name="outp", bufs=2) as outp,        # out staging
            tc.tile_pool(name="psum", bufs=1, space="PSUM") as psum,
            tc.tile_pool(name="dram", bufs=1, space="DRAM") as dramp,
        ):
            def ps_tile(tag, name, bufs):
                return psum.tile([P, 2 * SC], DT.float32, tag=tag, name=name,
                                 bufs=bufs)

            # ---------- phase 0: constants, slab kickstart ----------
            mask_sb = res.tile([P, NKT], DT.float32, tag="mask")
            nc.sync.dma_start(mask_sb[:], mask.ap().rearrange("(kt p) -> p kt", p=P))
            rel_sb = res.tile([NUM_BUCKETS, HL], DT.float32, tag="rel")
            nc.sync.dma_start(rel_sb[:], rel[:])

            # ediag one-hot tables first: tiny DMAs that feed the very first
            # PE work, ahead of the big slab/weight transfers
            oh_tiles = []
            for c in range(NDIAG // SC):
                oh_sb = stage.tile([NUM_BUCKETS, SC], DT.float32, tag="oh",
                                   name=f"oh{c}", bufs=4)
                nc.sync.dma_start(oh_sb[:], oh[:, c * SC:(c + 1) * SC])
                oh_tiles.append(oh_sb)
                if c == 3:
                    break

            # slab streaming: consumption order ph1 [0..3], then filler pairs
            slab_order = [nq for nq in range(NQC)] * 4
            slab_tiles = {}
            slab_next = [0]

            def emit_slab_dma():
                i = slab_next[0]
                if i >= len(slab_order):
                    return
                slab_next[0] = i + 1
                nq = slab_order[i]
                t = xsl.tile([P, NDT, SC], DT.bfloat16, tag="xs", name=f"xs{i}")
                nc.sync.dma_start(
                    t[:],
                    xt[:, nq * SC:(nq + 1) * SC].rearrange("(kt p) s -> p kt s", p=P),
                )
                slab_tiles[i] = t

            emit_slab_dma()   # slab for ph1 chunk 0
            emit_slab_dma()   # slab for ph1 chunk 1

            # weights (resident, bf16); wo streamed later via ubig
            wq_sb = res.tile([P, NDT, HL * D_KV], DT.bfloat16, tag="wq")
            wk_sb = res.tile([P, NDT, HL * D_KV], DT.bfloat16, tag="wk")
            wv_sb = res.tile([P, NDT, HL * D_KV], DT.bfloat16, tag="wv")
            nc.sync.dma_start(wq_sb[:], wq.ap().rearrange("(kt p) h -> p kt h", p=P))
            nc.sync.dma_start(wk_sb[:], wk.ap().rearrange("(kt p) h -> p kt h", p=P))
            nc.sync.dma_start(wv_sb[:], wv.ap().rearrange("(kt p) h -> p kt h", p=P))

            # persistent activations
            qt_sb = res.tile([P, NMT, S], DT.bfloat16, tag="qt")   # q REVERSED
            kt_sb = res.tile([P, NMT, S], DT.bfloat16, tag="kt")
            vaug = res.tile([P, NKT, HL, MV], DT.bfloat16, tag="vaug")
            ctxt = res.tile([P, NMT, S], DT.bfloat16, tag="ctxt")
            nc.vector.memset(vaug[:, :, :, D_KV:MV], 1.0)  # ones (denominator) col
            den_sb = res.tile([P, 2, SC], DT.bfloat16, tag="den")
            ones_sb = res.tile([P, D_KV], DT.bfloat16, tag="ones")
            nc.vector.memset(ones_sb[:], 1.0)

            # ---------- ediag: U tables to DRAM ----------
            ediag_dram = dramp.tile([HL, NDIAG], DT.bfloat16)
            den_dram = dramp.tile([2 * NMT * NQC, 2, SC], DT.bfloat16)
            rcp_dram = dramp.tile([2 * NMT * NQC, 2, SC], DT.bfloat16)
            for c in range(NDIAG // SC):
                if c < 4:
                    oh_sb = oh_tiles[c]
                else:
                    oh_sb = stage.tile([NUM_BUCKETS, SC], DT.float32, tag="oh",
                                       name=f"oh{c}", bufs=4)
                    nc.sync.dma_start(oh_sb[:], oh[:, c * SC:(c + 1) * SC])
                ed_ps = ps_tile("s", f"edps{c}", 2)[:HL, 0:SC]
                nc.tensor.matmul(ed_ps[:], rel_sb[:], oh_sb[:], start=True, stop=True)
                ed_sb = stage.tile([HL, SC], DT.bfloat16, tag="ed_sb",
                                   name=f"ed{c}")
                nc.scalar.activation(out=ed_sb[:], in_=ed_ps[:], func=AF.Exp)
                nc.sync.dma_start(ediag_dram[:, c * SC:(c + 1) * SC], ed_sb[:])

            def load_u(pr):
                """U pair tile [128, 2, W_U] for heads (2pr, 2pr+1)."""
                u = ubig.tile([P, 2, W_U], DT.bfloat16, tag="u", name=f"u{pr}")
                for i, hh in enumerate((2 * pr, 2 * pr + 1)):
                    shear = bass.AP(
                        tensor=ediag_dram.tensor,
                        offset=ediag_dram.offset + hh * NDIAG,
                        ap=[[1, P], [1, W_U]],
                    )
                    nc.sync.dma_start(u[:, i, :], shear)
                return u

            u_cur = load_u(0)

            # ---------- phase 1: per slab: pair-0 Q/K chunk + V chunk ----------
            def proj_qk_mms(pr, nq, slab, qk_ps, kd):
                """Emit the two projection matmuls for contraction step kd."""
                q_ps, k_ps = qk_ps[:, 0:SC], qk_ps[:, SC:2 * SC]
                nc.tensor.matmul(
                    q_ps, wq_sb[:, kd, pr * P:(pr + 1) * P], slab[:, kd, :],
                    start=(kd == 0), stop=(kd == NDT - 1),
                )
                nc.tensor.matmul(
                    k_ps, wk_sb[:, kd, pr * P:(pr + 1) * P], slab[:, kd, :],
                    start=(kd == 0), stop=(kd == NDT - 1),
                )

            def proj_qk_readout_q(pr, nq, qk_ps):
                dst = qt_sb[:, pr, :]
                rev = bass.AP(
                    tensor=dst.tensor,
                    offset=dst.offset + (S - 1 - nq * SC),
                    ap=[list(dst.ap[0]), [-1, SC]],
                )
                nc.vector.tensor_copy(rev, qk_ps[:, 0:SC])

            def proj_qk_readout_k(pr, nq, qk_ps):
                nc.vector.tensor_copy(kt_sb[:, pr, nq * SC:(nq + 1) * SC],
                                      qk_ps[:, SC:2 * SC])

            def proj_qk_readout(pr, nq, qk_ps):
                proj_qk_readout_q(pr, nq, qk_ps)
                proj_qk_readout_k(pr, nq, qk_ps)

            for nq in range(NQC):
                slab = slab_tiles[nq]
                emit_slab_dma()
                # pair-0 Q/K
                qk_ps = ps_tile("pj", f"qkps0_{nq}", 1)
                for kd in range(NDT):
                    proj_qk_mms(0, nq, slab, qk_ps, kd)
                proj_qk_readout(0, nq, qk_ps)
                # V (all heads)
                v_pair = [ps_tile("s", f"vps{nq}_{i}", 2) for i in range(2)]
                v_ps = [v_pair[0][:, 0:SC], v_pair[0][:, SC:2 * SC],
                        v_pair[1][:, 0:SC], v_pair[1][:, SC:2 * SC]]
                for kd in range(NDT):
                    for st in range(4):
                        nc.tensor.matmul(
                            v_ps[st], slab[:, kd, st * P:(st + 1) * P],
                            wv_sb[:, kd, :],
                            start=(kd == 0), stop=(kd == NDT - 1),
                        )
                for st in range(4):
                    kt_glob = nq * 4 + st
                    nc.vector.tensor_copy(
                        vaug[:, kt_glob, :, 0:D_KV],
                        v_ps[st].rearrange("p (h d) -> p h d", d=D_KV),
                    )

            # ---------- phase 2: attention with in-iteration PE filler ----------
            outp_ctr = [0]

            def out_group(st, ndp):
                """Output projection for one (st, nd-pair): 8 MMs with lhsT
                reused across the nd pair, then readout + DMA. Yields after
                each MM so attention can interleave."""
                o_ps = ps_tile("pj", f"ops{st}_{ndp}", 1)
                c0 = ndp * 2 * SC
                for m in range(NMT):
                    lh = ctxt[:, m, st * P:(st + 1) * P]
                    nc.tensor.matmul(o_ps[:, 0:SC], lh, wo_sb[:, m, c0:c0 + SC],
                                     start=(m == 0), stop=(m == NMT - 1))
                    yield
                    nc.tensor.matmul(o_ps[:, SC:2 * SC], lh,
                                     wo_sb[:, m, c0 + SC:c0 + 2 * SC],
                                     start=(m == 0), stop=(m == NMT - 1))
                    yield
                o_t = outp.tile([P, 2 * SC], DT.float32, tag="out",
                                name=f"ot{st}_{ndp}")
                k = outp_ctr[0]
                outp_ctr[0] += 1
                if k % 2 == 0:
                    nc.vector.tensor_copy(o_t[:], o_ps[:])
                else:
                    nc.scalar.copy(o_t[:], o_ps[:])
                nc.sync.dma_start(
                    out[st * P:(st + 1) * P, c0:c0 + 2 * SC], o_t[:]
                )

            def proj_filler(pr, qc, slab):
                """Generator: next pair's Q/K projection, 1 MM per step."""
                qk_ps = ps_tile("pj", f"qkps{pr}_{qc}", 1)
                q_ps, k_ps = qk_ps[:, 0:SC], qk_ps[:, SC:2 * SC]
                for kd in range(NDT):
                    nc.tensor.matmul(
                        q_ps, wq_sb[:, kd, pr * P:(pr + 1) * P], slab[:, kd, :],
                        start=(kd == 0), stop=(kd == NDT - 1),
                    )
                    yield
                    nc.tensor.matmul(
                        k_ps, wk_sb[:, kd, pr * P:(pr + 1) * P], slab[:, kd, :],
                        start=(kd == 0), stop=(kd == NDT - 1),
                    )
                    yield
                proj_qk_readout_q(pr, qc, qk_ps)
                yield
                proj_qk_readout_k(pr, qc, qk_ps)
                while True:
                    yield

            def outproj_filler(qc_range):
                """Generator: out-proj groups for st-range of qc_range,
                1 MM per step."""
                st0 = (NQC - 1 - qc_range) * 4
                for st in range(st0, st0 + 4):
                    for ndp in range(2):
                        yield from out_group(st, ndp)
                while True:
                    yield

            def null_filler():
                while True:
                    yield

            def emit_denchain(pr, qc):
                """Reciprocal + broadcast of the denominators via DRAM
                bounces: reshape [2,512] -> [128,8] so the DVE reciprocal's
                free-size (and thus cost, ~6.5ns/elem) is tiny, then
                broadcast the reciprocals across partitions (stride-0 DRAM
                reads). ~6us of pure-DMA latency, fully hidden by deferral."""
                blk = pr * NQC + qc
                nc.sync.dma_start(den_dram[blk, :, :], den_sb[64:65, :, :])
                base = den_dram[blk, 0, :]
                d8 = stage.tile([P, 8], DT.bfloat16, tag="d8",
                                name=f"d8_{pr}_{qc}", bufs=2)
                nc.sync.dma_start(
                    d8[:], bass.AP(tensor=base.tensor, offset=base.offset,
                                   ap=[[8, P], [1, 8]]))
                r8 = stage.tile([P, 8], DT.bfloat16, tag="r8",
                                name=f"r8_{pr}_{qc}", bufs=2)
                with nc.allow_low_precision(reason="bf16 softmax denom"):
                    nc.vector.reciprocal(r8[:], d8[:])
                rbase = rcp_dram[blk, 0, :]
                nc.sync.dma_start(
                    bass.AP(tensor=rbase.tensor, offset=rbase.offset,
                            ap=[[8, P], [1, 8]]), r8[:])
                rb_sb = stage.tile([P, SC], DT.bfloat16, tag="rb",
                                   name=f"rb{pr}_{qc}", bufs=2)
                for r in range(2):
                    src_row = rcp_dram[blk, r, :]
                    src = bass.AP(tensor=src_row.tensor, offset=src_row.offset,
                                  ap=[[0, 64], [1, SC]])
                    nc.sync.dma_start(rb_sb[r * 64:(r + 1) * 64, :], src)
                return rb_sb

            def emit_norm(pr, qc, rb_sb):
                """Rescale ctxt by the broadcast reciprocals (one DVE mult)."""
                q0t = S - (qc + 1) * SC
                cslc = ctxt[:, pr, q0t:q0t + SC]
                nc.vector.tensor_tensor(cslc, cslc, rb_sb[:], OP.mult)

            def emit_norm_now(pr, qc):
                """Immediate normalization (pair 3): K=1 PE broadcast of the
                denominator rows, parallel reciprocal, rescale. Low latency so
                the following block's out-proj filler is not held up."""
                rb_ps = ps_tile("pj", f"rbp{pr}_{qc}", 1)
                for r in range(2):
                    nc.tensor.matmul(
                        rb_ps[r * 64:(r + 1) * 64, 0:SC],
                        ones_sb[64:65, :], den_sb[64:65, r, :],
                        start=True, stop=True,
                    )
                rcp = stage.tile([P, SC], DT.bfloat16, tag="rcp",
                                 name=f"rcpn{pr}_{qc}", bufs=2)
                with nc.allow_low_precision(reason="bf16 softmax denom"):
                    nc.vector.reciprocal(rcp[:], rb_ps[:, 0:SC])
                q0t = S - (qc + 1) * SC
                cslc = ctxt[:, pr, q0t:q0t + SC]
                nc.vector.tensor_tensor(cslc, cslc, rcp[:], OP.mult)

            def attn_qc(pr, qc, u_t, filler, steps_per_iter, pending_norm):
                """Attention for head pair pr, reversed-q chunk qc.
                pending_norm: previous block's normalization closure; emitted
                at block start for pair 3 (out-proj filler depends on it) or
                after iteration 1 otherwise (hides the rcp data latency).
                Filler steps are front-loaded (2 iterations' worth before the
                first QK) to cover the block-boundary psum-slot waits."""
                jg0 = qc * SC
                cx01 = ps_tile("cx", f"cx{pr}_{qc}", 1)
                cx0, cx1 = cx01[0:MV, 0:SC], cx01[0:MV, SC:2 * SC]

                def emit_qk(kt):
                    s01 = ps_tile("s", f"s{pr}_{qc}_{kt}", 2)
                    nc.tensor.matmul(
                        s01[:, 0:SC], kt_sb[0:64, pr, kt * P:(kt + 1) * P],
                        qt_sb[0:64, pr, jg0:jg0 + SC],
                        start=True, stop=True, tile_position=(0, 0),
                    )
                    nc.tensor.matmul(
                        s01[:, SC:2 * SC], kt_sb[64:128, pr, kt * P:(kt + 1) * P],
                        qt_sb[64:128, pr, jg0:jg0 + SC],
                        start=True, stop=True, tile_position=(64, 0),
                    )
                    return s01

                s01 = emit_qk(0)
                # small front-load to cover the block-boundary latencies
                for _ in range(steps_per_iter):
                    next(filler)
                for kt in range(NKT):
                    s01_next = emit_qk(kt + 1) if kt + 1 < NKT else None
                    px = pexpp.tile([P, 2 * SC], DT.bfloat16, tag="pexp",
                                    name=f"px{pr}_{qc}_{kt}")
                    nc.scalar.activation(
                        out=px[:], in_=s01[:], func=AF.Exp,
                        bias=mask_sb[:, kt:kt + 1], scale=1.0 / math.sqrt(D_KV),
                    )
                    j0 = kt * P + jg0
                    nc.vector.tensor_tensor(
                        px[:], px[:], u_t[:, :, j0:j0 + SC], OP.mult
                    )
                    if kt < NKT - 1:
                        for _ in range(steps_per_iter):
                            next(filler)
                    elif steps_per_iter:
                        # last iter: exhaust the generator so the filler's
                        # readout (frees its psum slot) is emitted in-block
                        for _ in range(2):
                            next(filler)
                    nc.tensor.matmul(
                        cx0, vaug[:, kt, 2 * pr, :], px[:, 0:SC],
                        start=(kt == 0), stop=(kt == NKT - 1),
                    )
                    nc.tensor.matmul(
                        cx1, vaug[:, kt, 2 * pr + 1, :], px[:, SC:2 * SC],
                        start=(kt == 0), stop=(kt == NKT - 1),
                    )
                    s01 = s01_next
                    if kt == 8 and pending_norm is not None:
                        pending_norm()
                        pending_norm = None

                # readout: un-reversed unnormalized ctx (heads at part 0:64/64:128)
                # ctxt copies on DVE; the single merged denominator copy on ACT
                # (so the next block's exps are minimally delayed on either).
                for r, cx in ((0, cx0), (1, cx1)):
                    base = ctxt[r * 64:r * 64 + 64, pr, :]
                    dst = bass.AP(
                        tensor=base.tensor,
                        offset=base.offset + (S - 1 - jg0),
                        ap=[list(base.ap[0]), [-1, SC]],
                    )
                    nc.vector.tensor_copy(dst, cx[0:D_KV, :])
                if pr < 3:
                    # denominator rows to SBUF reversed; h0 on ACT (cheap,
                    # ahead of next block's exps), h1 on DVE
                    nc.scalar.copy(_rev_free(den_sb[64:65, 0, :], SC),
                                   cx0[D_KV:MV, :])
                    nc.vector.tensor_copy(_rev_free(den_sb[64:65, 1, :], SC),
                                          cx1[D_KV:MV, :])
                    return emit_denchain(pr, qc)
                dsl = den_sb[64:65, 0, :]
                dden = bass.AP(
                    tensor=dsl.tensor,
                    offset=dsl.offset + (SC - 1),
                    ap=[list(dsl.ap[0]), [SC, 2], [-1, SC]],
                )
                nc.scalar.copy(dden, cx01[D_KV:MV, 0:2 * SC])
                return None

            wo_sb = None
            pending = None
            for pr in range(HL // 2):
                if pr < 3:
                    u_next = load_u(pr + 1)
                else:
                    wo_sb = ubig.tile([P, NMT, D], DT.bfloat16, tag="u", name="wo")
                    nc.sync.dma_start(
                        wo_sb[:], wo.ap().rearrange("(mt p) d -> p mt d", p=P))
                for qc in range(NQC):
                    if pr < 3:
                        slab = slab_tiles[4 + pr * 4 + qc]
                        emit_slab_dma()
                        filler = proj_filler(pr + 1, qc, slab)
                        steps = 2
                    elif qc > 0:
                        filler = outproj_filler(qc - 1)
                        steps = 4
                    else:
                        filler = null_filler()
                        steps = 0
                    rb_sb = attn_qc(pr, qc, u_cur, filler, steps, pending)
                    if pr < 3 or qc > 0:
                        # drain remaining filler-side emissions (readout etc.)
                        for _ in range(8):
                            next(filler)
                    if pr < 3:
                        pending = (lambda p=pr, q=qc, rb=rb_sb:
                                   emit_norm(p, q, rb))
                    else:
                        emit_norm_now(pr, qc)
                        pending = None
                if pr < 3:
                    u_cur = u_next

            # ---------- phase 3: remaining out-proj (range of qc=3) ----------
            tags = ["pj", "s", "cx"]
            for st in range(0, 4):
                for ndp in range(2):
                    tg = tags[(st * 2 + ndp) % 3]
                    o_ps = psum.tile([P, 2 * SC], DT.float32, tag=tg,
                                     name=f"tl{st}_{ndp}",
                                     bufs=(2 if tg == "s" else 1))
                    c0 = ndp * 2 * SC
                    for m in range(NMT):
                        lh = ctxt[:, m, st * P:(st + 1) * P]
                        nc.tensor.matmul(o_ps[:, 0:SC], lh,
                                         wo_sb[:, m, c0:c0 + SC],
                                         start=(m == 0), stop=(m == NMT - 1))
                        nc.tensor.matmul(o_ps[:, SC:2 * SC], lh,
                                         wo_sb[:, m, c0 + SC:c0 + 2 * SC],
                                         start=(m == 0), stop=(m == NMT - 1))
                    o_t = outp.tile([P, 2 * SC], DT.float32, tag="out",
                                    name=f"otl{st}_{ndp}")
                    if (st * 2 + ndp) % 2 == 0:
                        nc.vector.tensor_copy(o_t[:], o_ps[:])
                    else:
                        nc.scalar.copy(o_t[:], o_ps[:])
                    nc.sync.dma_start(
                        out[st * P:(st + 1) * P, c0:c0 + 2 * SC], o_t[:]
                    )

    nc.finalize()
    return nc


_NC_CACHE = None


def _get_nc():
    global _NC_CACHE
    if _NC_CACHE is None:
        _NC_CACHE = _build()
    return _NC_CACHE


def _in_maps(hidden_states, attention_mask, Wq, Wk, Wv, Wo, rel_emb):
    oh = _onehot_const()
    import ml_dtypes
    bf16 = ml_dtypes.bfloat16
    maps = []
    for c in range(NCORES):
        b, g = c // 4, c % 4
        hlo, hhi = g * HL, (g + 1) * HL
        maps.append({
            "xt": np.ascontiguousarray(hidden_states[b].T).astype(bf16),
            "wq": np.ascontiguousarray(Wq[:, hlo * D_KV:hhi * D_KV]).astype(bf16),
            "wk": np.ascontiguousarray(Wk[:, hlo * D_KV:hhi * D_KV]).astype(bf16),
            "wv": np.ascontiguousarray(Wv[:, hlo * D_KV:hhi * D_KV]).astype(bf16),
            "wo": np.ascontiguousarray(Wo[hlo * D_KV:hhi * D_KV, :]).astype(bf16),
            "mask": np.ascontiguousarray(attention_mask[b, 0, 0, :]).astype(np.float32),
            "rel": np.ascontiguousarray(rel_emb[:, hlo:hhi]).astype(np.float32),
            "oh": oh,
        })
    return maps


def kernel(hidden_states, attention_mask, Wq, Wk, Wv, Wo, rel_emb, _trace=False,
           _trace_kwargs=None):
    hidden_states = np.asarray(hidden_states, dtype=np.float32)
    attention_mask = np.asarray(attention_mask, dtype=np.float32)
    Wq = np.asarray(Wq, dtype=np.float32)
    Wk = np.asarray(Wk, dtype=np.float32)
    Wv = np.asarray(Wv, dtype=np.float32)
    Wo = np.asarray(Wo, dtype=np.float32)
    rel_emb = np.asarray(rel_emb, dtype=np.float32)

    nc = _get_nc()
    maps = _in_maps(hidden_states, attention_mask, Wq, Wk, Wv, Wo, rel_emb)
    kw = dict(_trace_kwargs or {})
    res = run_bass_kernel_spmd(nc, maps, core_ids=list(range(NCORES)),
                               trace=_trace, **kw)
    kernel.last_results = res
    outp = np.empty((B, S, D), dtype=np.float32)
    for b in range(B):
        acc = np.asarray(res.results[4 * b]["out"], dtype=np.float32).copy()
        for g in range(1, 4):
            acc += np.asarray(res.results[4 * b + g]["out"], dtype=np.float32)
        outp[b] = acc
    return outp
